# revision 32
# baseline (speedup 1.0000x reference)
"""Multi-head attention TRN2 kernel (8 NeuronCores, SPMD).

Problem: B=2, N=2048, D=1024, H=16 heads of dim 64, fp32, per-(b,h)
key-length masking (valid_len, length 32).

Sharding: batch*heads across 8 cores - core c handles batch b=c//4 and 4
heads ("slots", rank-aligned by valid_len so the SPMD trip counts stay
balanced).  Per core:

  phase P (projections, bf16 inputs to halve HBM traffic):
    K^T/Q^T = Wslice^T @ x^T   (head dims on partitions, positions free)
    V       = x^T-tiles as lhsT, Wv as rhs  (positions on partitions),
              copied into V1 = [V_j | 1] blocks per (key-tile, slot)
  phase A (attention, f32r):
    S^T   = K^T.T @ Q^T per (slot, key-tile), batched in PSUM pairs
    P^T   = exp(S^T/8 + bias) on ScalarE - the valid_len mask is a
            per-partition bias column (0 / -30000), so no V masking ops;
            key-tile pairs that are uniformly valid/invalid across all
            cores share one exp instruction ([128,1024] batch)
    acc   = V1.T @ P^T accumulated over key tiles (ones column gives the
            softmax denominator as row 64)
    normalize via DVE reciprocal + Newton, GpSimd partition broadcast
    out_partial = heads^T.T @ Wo_slice -> (2048, 1024) per core
Host sums the 4 per-core partials of each batch element (the unshard for
the row-sharded Wo) and gathers.

Matmuls: projections run bf16*bf16 (f32 accumulate); attention runs
float32r (full PE rate at free>=256).
"""
import sys
import numpy as np
from contextlib import ExitStack

sys.path.insert(0, "/opt/trn_rl_repo")

import concourse.bass as bass  # noqa: E402
from concourse import bacc, mybir  # noqa: E402
import concourse.tile as tile  # noqa: E402
from concourse.bass_utils import run_bass_kernel_spmd  # noqa: E402

F32 = mybir.dt.float32
F32R = mybir.dt.float32r
BF16 = mybir.dt.bfloat16
AF = mybir.ActivationFunctionType
NPBF16 = mybir.dt.np(BF16)

B, N, D, H = 2, 2048, 1024, 16
DH = 64
HPC = 4          # heads (slots) per core
NCORES = 8
QC = 512         # q chunk (matmul free dim)
NKT = N // 128   # 16 k tiles
NDC = D // 128   # 8 contraction chunks
MASK_BIAS = -30000.0

LAST_RESULTS = None  # BassKernelResults of the most recent run (for tooling)


def _build_program(trips, plans):
    """trips: 4 ints (k-tile count per slot); plans: per slot, list of
    (t0, ntiles) exp-batch groups covering range(trips[j])."""
    nc = bacc.Bacc("TRN2", target_bir_lowering=False, debug=False,
                   num_devices=NCORES)

    xTq = nc.dram_tensor("xTq", [D, N], BF16, kind="ExternalInput")
    xTk = nc.dram_tensor("xTk", [D, N], BF16, kind="ExternalInput")
    xTv = nc.dram_tensor("xTv", [D, N], BF16, kind="ExternalInput")
    wq = nc.dram_tensor("wq", [128, NDC * 256], BF16, kind="ExternalInput")
    wk = nc.dram_tensor("wk", [128, NDC * 256], BF16, kind="ExternalInput")
    wv = nc.dram_tensor("wv", [128, NDC * 256], BF16, kind="ExternalInput")
    wo = nc.dram_tensor("wo", [256, D], BF16, kind="ExternalInput")
    vmask = nc.dram_tensor("vmask", [128, HPC * NKT], F32, kind="ExternalInput")
    out = nc.dram_tensor("out", [N, D], F32, kind="ExternalOutput")

    with tile.TileContext(nc) as tc:
        with ExitStack() as ctx:
            wpool = ctx.enter_context(tc.tile_pool(name="wpool", bufs=1))
            xpool = ctx.enter_context(tc.tile_pool(name="xpool", bufs=6))
            qkpool = ctx.enter_context(tc.tile_pool(name="qkpool", bufs=1))
            v1pool = ctx.enter_context(tc.tile_pool(name="v1pool", bufs=1))
            ptpool = ctx.enter_context(tc.tile_pool(name="ptpool", bufs=8))
            nrmpool = ctx.enter_context(tc.tile_pool(name="nrmpool", bufs=4))
            pbpool = ctx.enter_context(tc.tile_pool(name="pbpool", bufs=1))
            opool = ctx.enter_context(tc.tile_pool(name="opool", bufs=8))

            # only wk is needed before the first matmul; the other weight
            # loads are issued just before their consuming phase so they
            # don't delay the first xk chunks
            t_wk = wpool.tile([128, NDC * 256], BF16, tag="wk")
            t_wq = wpool.tile([128, NDC * 256], BF16, tag="wq")
            t_wv = wpool.tile([128, NDC * 256], BF16, tag="wv")
            t_wo = [wpool.tile([128, D], BF16, tag=f"wo{p}", name=f"t_wo{p}")
                    for p in range(2)]
            t_vm = wpool.tile([128, HPC * NKT], F32, tag="vm")

            # K^T/Q^T: [128 dims (2 slots), N] per slot-pair
            t_kT = [qkpool.tile([128, N], BF16, tag=f"kT{p}", name=f"t_kT{p}")
                    for p in range(2)]
            t_qT = [qkpool.tile([128, N], BF16, tag=f"qT{p}", name=f"t_qT{p}")
                    for p in range(2)]
            # V1: per key-tile t, 4 blocks of [V_j (64 cols) | ones (1 col)]
            t_v1 = v1pool.tile([128, NKT * HPC * 65], BF16, tag="v1")
            # normalized heads^T per slot pair: [128 dims, N]
            t_pb = [pbpool.tile([128, N], BF16, tag=f"pb{p}", name=f"t_pb{p}")
                    for p in range(2)]

            # ones columns of V1, one strided memset
            ones_ap = t_v1[:].rearrange(
                "p (b c) -> p b c", c=65)[:, :, 64:65]
            nc.vector.memset(ones_ap, 1.0)

            # ---- phase P: projections (K, Q, V) ----
            with tc.tile_pool(name="pp", bufs=8, space="PSUM") as pp:
                for si, (xin, wsb, dsts) in enumerate(
                        ((xTk, t_wk, t_kT), (xTq, t_wq, t_qT))):
                    accs = [pp.tile([128, QC], F32, tag="acc", name=f"acc_{i}")
                            for i in range(8)]
                    for c in range(NDC):
                        xt = xpool.tile([128, N], BF16, tag="xt")
                        nc.sync.dma_start(xt[:], xin[c * 128:(c + 1) * 128, :])
                        if si == 0:
                            # wk sliced per chunk: the first matmul only
                            # gates on 64KB of weights + one x chunk
                            nc.sync.dma_start(
                                t_wk[:, c * 256:(c + 1) * 256],
                                wk[:, c * 256:(c + 1) * 256])
                        for m in range(2):
                            for qq in range(4):
                                nc.tensor.matmul(
                                    accs[m * 4 + qq][:],
                                    wsb[:, c * 256 + m * 128:
                                        c * 256 + (m + 1) * 128],
                                    xt[:, qq * QC:(qq + 1) * QC],
                                    start=(c == 0), stop=(c == NDC - 1))
                    if si == 0:
                        nc.sync.dma_start(t_wq[:], wq[:])
                    else:
                        nc.sync.dma_start(t_wv[:], wv[:])
                    # drain PSUM->SBUF casts on BOTH ScalarE and DVE, q-chunk
                    # 0 first: the first attention S matmuls gate only on the
                    # chunk-0 casts, so phase A starts ~2.5us earlier
                    for n, i in enumerate((0, 4, 1, 5, 2, 6, 3, 7)):
                        dst = dsts[i // 4][:, (i % 4) * QC:(i % 4 + 1) * QC]
                        with nc.allow_low_precision(reason="f32r 4B"):
                            if n % 2 == 0:
                                nc.scalar.activation(dst, accs[i][:], AF.Copy)
                            else:
                                nc.vector.tensor_copy(dst, accs[i][:])
                nc.sync.dma_start(t_wo[0][:], wo[0:128, :])
                nc.sync.dma_start(t_wo[1][:], wo[128:256, :])
                nc.sync.dma_start(t_vm[:], vmask[:])
                # V projection: two half-column passes of 8 k-tiles each,
                # packed two k-tiles per PSUM bank (4 banks per pass) so
                # half the PSUM is free for the first attention S matmuls
                # to overlap the final V pass
                for g in range(2):
                    accs = [pp.tile([128, 512], F32, tag="acc",
                                    name=f"accv_{i}") for i in range(4)]
                    for c in range(NDC):
                        xt = xpool.tile([128, 1024], BF16, tag="xtv")
                        nc.sync.dma_start(
                            xt[:], xTv[c * 128:(c + 1) * 128,
                                       g * 1024:(g + 1) * 1024])
                        for kt8 in range(8):
                            # one accumulation group per bank: start zeroes
                            # the whole bank, the second column-half's first
                            # write lands via per-element has_written
                            nc.tensor.matmul(
                                accs[kt8 // 2][:, (kt8 % 2) * 256:
                                               (kt8 % 2 + 1) * 256],
                                xt[:, kt8 * 128:(kt8 + 1) * 128],
                                t_wv[:, c * 256:(c + 1) * 256],
                                start=(c == 0 and kt8 % 2 == 0),
                                stop=(c == NDC - 1 and kt8 % 2 == 1))
                    for kt8 in range(8):
                        t = g * 8 + kt8
                        # [128, 4, 64] strided copy: slot j -> V1 block
                        src = accs[kt8 // 2][:, (kt8 % 2) * 256:
                                             (kt8 % 2 + 1) * 256].rearrange(
                            "p (j c) -> p j c", c=64)
                        dst = t_v1[:, t * 260:(t + 1) * 260].rearrange(
                            "p (j c) -> p j c", c=65)[:, :, 0:64]
                        with nc.allow_low_precision(reason="f32r 4B"):
                            nc.vector.tensor_copy(dst, src)

            # ---- phase A: attention with fused output projection ----
            with tc.tile_pool(name="ap", bufs=1, space="PSUM") as ap:
                def emit_outproj(q):
                    # output projection for the 4 q-tiles of chunk q;
                    # accumulate pair 1 (shorter-plan slots) first so the
                    # first o_ps matmul fires before pair 0 is normalized
                    for qt in range(q * (QC // 128), (q + 1) * (QC // 128)):
                        ts = slice(qt * 128, (qt + 1) * 128)
                        stage = opool.tile([128, D], F32, tag="ostage")
                        for ch in range(2):
                            o_ps = ap.tile([128, 512], F32, tag="sT", bufs=2)
                            for p2 in (1, 0):
                                nc.tensor.matmul(
                                    o_ps[:], t_pb[p2][:, ts],
                                    t_wo[p2][:, ch * 512:(ch + 1) * 512],
                                    start=(p2 == 1), stop=(p2 == 0))
                            nc.any.tensor_copy(
                                stage[:, ch * 512:(ch + 1) * 512], o_ps[:])
                        nc.sync.dma_start(out[ts, :], stage[:])

                for q in range(N // QC):
                    qs = slice(q * QC, (q + 1) * QC)
                    accs2 = [ap.tile([65, QC], F32, tag="acc2", bufs=4,
                                     name=f"acc_{j}") for j in range(HPC)]
                    # round-robin the exp-batch groups across slots so the
                    # in-flight S->exp->PV chains are independent and PE
                    # always has a matmul ready while ScalarE runs exp
                    sched = []
                    for j in range(HPC):
                        for gi, g in enumerate(plans[j]):
                            sched.append((j, gi, g))
                    sched.sort(key=lambda x: (x[1], x[0]))
                    for gidx, (j, gi, (t0, nt)) in enumerate(sched):
                        if gidx == 4 and q > 0:
                            # previous chunk's output projection, emitted a
                            # few groups in so its PSUM slots don't block
                            # this chunk's S matmuls during the norm tail
                            emit_outproj(q - 1)
                        p, half = j // 2, j % 2
                        rows = slice(half * 64, (half + 1) * 64)
                        acc = accs2[j]
                        sT = ap.tile([128, nt * QC], F32, tag="sT", bufs=2)
                        for i in range(nt):
                            t = t0 + i
                            nc.tensor.matmul(
                                sT[:, i * QC:(i + 1) * QC],
                                t_kT[p][rows, t * 128:(t + 1) * 128],
                                t_qT[p][rows, qs],
                                start=True, stop=True)
                        pT = ptpool.tile([128, nt * QC], BF16, tag="pT")
                        nc.scalar.activation(
                            pT[:], sT[:], AF.Exp, scale=0.125,
                            bias=t_vm[:, j * NKT + t0: j * NKT + t0 + 1])
                        for i in range(nt):
                            t = t0 + i
                            base = (t * HPC + j) * 65
                            nc.tensor.matmul(
                                acc[:], t_v1[:, base: base + 65],
                                pT[:, i * QC:(i + 1) * QC],
                                start=(gi == 0 and i == 0),
                                stop=(gi == len(plans[j]) - 1 and i == nt - 1))
                        if gi == len(plans[j]) - 1:
                            # normalize this slot as soon as its last PV is
                            # in: 1/denom (DVE), partition-broadcast
                            # (GpSimd), scale (DVE)
                            r1 = nrmpool.tile([1, QC], F32, tag="r1")
                            nc.vector.reciprocal(r1[:], acc[64:65, :])
                            bc_sb = nrmpool.tile([64, QC], F32, tag="bc_sb")
                            nc.gpsimd.partition_broadcast(bc_sb[:], r1[:])
                            with nc.allow_low_precision(reason="f32r 4B"):
                                nc.vector.tensor_mul(
                                    t_pb[p][rows, qs], acc[0:64, :],
                                    bc_sb[:])
                emit_outproj(N // QC - 1)

    nc.finalize()
    return nc


def _make_plans(trips, vls_by_slot):
    """Greedy pair batching: (t, t+1) share one exp iff every core's vl is
    outside the open interval (128*t, 128*(t+2)) - then one bias column
    describes both tiles on every core."""
    plans = []
    for j in range(HPC):
        plan, t = [], 0
        while t < trips[j]:
            if t + 1 < trips[j] and all(
                    v <= 128 * t or v >= 128 * (t + 2)
                    for v in vls_by_slot[j]):
                plan.append((t, 2))
                t += 2
            else:
                plan.append((t, 1))
                t += 1
        plans.append(plan)
    return plans


def kernel(queries, keys, values, valid_len, Wq, Wk, Wv, Wo):
    global LAST_RESULTS
    queries = np.asarray(queries, dtype=np.float32)
    keys = np.asarray(keys, dtype=np.float32)
    values = np.asarray(values, dtype=np.float32)
    Wq = np.asarray(Wq, dtype=np.float32)
    Wk = np.asarray(Wk, dtype=np.float32)
    Wv = np.asarray(Wv, dtype=np.float32)
    Wo = np.asarray(Wo, dtype=np.float32)
    vl = np.asarray(valid_len).astype(np.int64).reshape(B * H)

    # rank-aligned slot assignment: per batch, heads sorted by vl desc;
    # slot j of the 4 cores of that batch takes ranks 4j..4j+3
    order = {}
    for b in range(B):
        idx = (np.argsort(-vl[b * H:(b + 1) * H], kind="stable") + b * H)
        for cg in range(4):
            order[b * 4 + cg] = [int(idx[4 * j + cg]) for j in range(HPC)]
    trips, vls_by_slot = [], []
    for j in range(HPC):
        vs = [int(vl[order[c][j]]) for c in range(NCORES)]
        vls_by_slot.append(vs)
        m = max(-(-v // 128) for v in vs)
        trips.append(max(1, min(NKT, m)))
    plans = _make_plans(trips, vls_by_slot)

    nc = _build_program(tuple(trips), plans)

    in_maps = []
    for c in range(NCORES):
        b = c // 4
        heads = order[c]
        cols = np.concatenate(
            [np.arange((h - b * H) * DH, (h - b * H + 1) * DH) for h in heads])

        def wlayout(w):
            return np.ascontiguousarray(
                w[:, cols].reshape(NDC, 128, 256).transpose(1, 0, 2)
                .reshape(128, NDC * 256).astype(NPBF16))

        vm = np.zeros((128, HPC * NKT), np.float32)
        for j, h in enumerate(heads):
            bias = np.where(np.arange(N) < vl[h], 0.0, MASK_BIAS)
            vm[:, j * NKT:(j + 1) * NKT] = bias.reshape(NKT, 128).T

        in_maps.append({
            "xTq": np.ascontiguousarray(queries[b].T.astype(NPBF16)),
            "xTk": np.ascontiguousarray(keys[b].T.astype(NPBF16)),
            "xTv": np.ascontiguousarray(values[b].T.astype(NPBF16)),
            "wq": wlayout(Wq),
            "wk": wlayout(Wk),
            "wv": wlayout(Wv),
            "wo": np.ascontiguousarray(Wo[cols, :]).astype(NPBF16),
            "vmask": vm,
        })

    LAST_RESULTS = run_bass_kernel_spmd(nc, in_maps, list(range(NCORES)))
    res = LAST_RESULTS.results

    out = np.zeros((B, N, D), np.float64)
    for c in range(NCORES):
        out[c // 4] += res[c]["out"].astype(np.float64)
    return out.astype(np.float32)


# revision 35
# speedup vs baseline: 1.0044x; 1.0044x over previous
"""Multi-head attention TRN2 kernel (8 NeuronCores, SPMD).

Problem: B=2, N=2048, D=1024, H=16 heads of dim 64, fp32, per-(b,h)
key-length masking (valid_len, length 32).

Sharding: batch*heads across 8 cores - core c handles batch b=c//4 and 4
heads ("slots", rank-aligned by valid_len so the SPMD trip counts stay
balanced).  Per core:

  phase P (projections, bf16 inputs to halve HBM traffic):
    K^T/Q^T = Wslice^T @ x^T   (head dims on partitions, positions free)
    V       = x^T-tiles as lhsT, Wv as rhs  (positions on partitions),
              copied into V1 = [V_j | 1] blocks per (key-tile, slot)
  phase A (attention, all-bf16 operands, f32 PSUM accumulate):
    S^T   = K^T.T @ Q^T per (slot, key-tile), batched in PSUM pairs
    P^T   = exp(S^T/8 + bias) on ScalarE - the valid_len mask is a
            per-partition bias column (0 / -30000), so no V masking ops;
            key-tile pairs that are uniformly valid/invalid across all
            cores share one exp instruction ([128,1024] batch); groups
            are emitted round-robin across slots so independent
            S->exp->PV chains keep the tensor engine dense
    acc   = V1.T @ P^T accumulated over key tiles (ones column gives the
            softmax denominator as row 64)
    normalize per slot right after its last PV: DVE bit-exact
    reciprocal, GpSimd partition broadcast, DVE scale
    out_partial = heads^T.T @ Wo_slice -> (2048, 1024) per core; each
    chunk's output projection is emitted a few groups into the NEXT
    chunk so its PSUM slots never starve the S-matmul pipeline
Host sums the 4 per-core partials of each batch element (the unshard for
the row-sharded Wo) and gathers.
"""
import sys
import numpy as np
from contextlib import ExitStack

sys.path.insert(0, "/opt/trn_rl_repo")

import concourse.bass as bass  # noqa: E402
from concourse import bacc, mybir  # noqa: E402
import concourse.tile as tile  # noqa: E402
from concourse.bass_utils import run_bass_kernel_spmd  # noqa: E402

F32 = mybir.dt.float32
F32R = mybir.dt.float32r
BF16 = mybir.dt.bfloat16
AF = mybir.ActivationFunctionType
NPBF16 = mybir.dt.np(BF16)

B, N, D, H = 2, 2048, 1024, 16
DH = 64
HPC = 4          # heads (slots) per core
NCORES = 8
QC = 512         # q chunk (matmul free dim)
NKT = N // 128   # 16 k tiles
NDC = D // 128   # 8 contraction chunks
MASK_BIAS = -30000.0

LAST_RESULTS = None  # BassKernelResults of the most recent run (for tooling)


def _build_program(trips, plans):
    """trips: 4 ints (k-tile count per slot); plans: per slot, list of
    (t0, ntiles) exp-batch groups covering range(trips[j])."""
    nc = bacc.Bacc("TRN2", target_bir_lowering=False, debug=False,
                   num_devices=NCORES)

    xTq = nc.dram_tensor("xTq", [D, N], BF16, kind="ExternalInput")
    xTk = nc.dram_tensor("xTk", [D, N], BF16, kind="ExternalInput")
    xTv = nc.dram_tensor("xTv", [D, N], BF16, kind="ExternalInput")
    wq = nc.dram_tensor("wq", [128, NDC * 256], BF16, kind="ExternalInput")
    wk = nc.dram_tensor("wk", [128, NDC * 256], BF16, kind="ExternalInput")
    wv = nc.dram_tensor("wv", [128, NDC * 256], BF16, kind="ExternalInput")
    wo = nc.dram_tensor("wo", [256, D], BF16, kind="ExternalInput")
    vmask = nc.dram_tensor("vmask", [128, HPC * NKT], F32, kind="ExternalInput")
    out = nc.dram_tensor("out", [N, D], F32, kind="ExternalOutput")

    with tile.TileContext(nc) as tc:
        with ExitStack() as ctx:
            wpool = ctx.enter_context(tc.tile_pool(name="wpool", bufs=1))
            xpool = ctx.enter_context(tc.tile_pool(name="xpool", bufs=6))
            qkpool = ctx.enter_context(tc.tile_pool(name="qkpool", bufs=1))
            v1pool = ctx.enter_context(tc.tile_pool(name="v1pool", bufs=1))
            ptpool = ctx.enter_context(tc.tile_pool(name="ptpool", bufs=8))
            nrmpool = ctx.enter_context(tc.tile_pool(name="nrmpool", bufs=4))
            pbpool = ctx.enter_context(tc.tile_pool(name="pbpool", bufs=1))
            opool = ctx.enter_context(tc.tile_pool(name="opool", bufs=8))

            # only wk is needed before the first matmul; the other weight
            # loads are issued just before their consuming phase so they
            # don't delay the first xk chunks
            t_wk = wpool.tile([128, NDC * 256], BF16, tag="wk")
            t_wq = wpool.tile([128, NDC * 256], BF16, tag="wq")
            t_wv = wpool.tile([128, NDC * 256], BF16, tag="wv")
            t_wo = [wpool.tile([128, D], BF16, tag=f"wo{p}", name=f"t_wo{p}")
                    for p in range(2)]
            t_vm = wpool.tile([128, HPC * NKT], F32, tag="vm")

            # K^T/Q^T: [128 dims (2 slots), N] per slot-pair
            t_kT = [qkpool.tile([128, N], BF16, tag=f"kT{p}", name=f"t_kT{p}")
                    for p in range(2)]
            t_qT = [qkpool.tile([128, N], BF16, tag=f"qT{p}", name=f"t_qT{p}")
                    for p in range(2)]
            # V1: per key-tile t, 4 blocks of [V_j (64 cols) | ones (1 col)]
            t_v1 = v1pool.tile([128, NKT * HPC * 65], BF16, tag="v1")
            # normalized heads^T per slot pair: [128 dims, N]
            t_pb = [pbpool.tile([128, N], BF16, tag=f"pb{p}", name=f"t_pb{p}")
                    for p in range(2)]

            # ones columns of V1, one strided memset
            ones_ap = t_v1[:].rearrange(
                "p (b c) -> p b c", c=65)[:, :, 64:65]
            nc.vector.memset(ones_ap, 1.0)

            # ---- phase P: projections (K, Q, V) ----
            with tc.tile_pool(name="pp", bufs=8, space="PSUM") as pp:
                for si, (xin, wsb, dsts) in enumerate(
                        ((xTk, t_wk, t_kT), (xTq, t_wq, t_qT))):
                    accs = [pp.tile([128, QC], F32, tag="acc", name=f"acc_{i}")
                            for i in range(8)]
                    for c in range(NDC):
                        xt = xpool.tile([128, N], BF16, tag="xt")
                        nc.sync.dma_start(xt[:], xin[c * 128:(c + 1) * 128, :])
                        if si == 0:
                            # wk sliced per chunk: the first matmul only
                            # gates on 64KB of weights + one x chunk
                            nc.sync.dma_start(
                                t_wk[:, c * 256:(c + 1) * 256],
                                wk[:, c * 256:(c + 1) * 256])
                        for m in range(2):
                            for qq in range(4):
                                nc.tensor.matmul(
                                    accs[m * 4 + qq][:],
                                    wsb[:, c * 256 + m * 128:
                                        c * 256 + (m + 1) * 128],
                                    xt[:, qq * QC:(qq + 1) * QC],
                                    start=(c == 0), stop=(c == NDC - 1))
                    if si == 0:
                        nc.sync.dma_start(t_wq[:], wq[:])
                    else:
                        nc.sync.dma_start(t_wv[:], wv[:])
                    # drain PSUM->SBUF casts on BOTH ScalarE and DVE, q-chunk
                    # 0 first: the first attention S matmuls gate only on the
                    # chunk-0 casts, so phase A starts ~2.5us earlier
                    for n, i in enumerate((0, 4, 1, 5, 2, 6, 3, 7)):
                        dst = dsts[i // 4][:, (i % 4) * QC:(i % 4 + 1) * QC]
                        with nc.allow_low_precision(reason="f32r 4B"):
                            if n % 2 == 0:
                                nc.scalar.activation(dst, accs[i][:], AF.Copy)
                            else:
                                nc.vector.tensor_copy(dst, accs[i][:])
                nc.sync.dma_start(t_wo[0][:], wo[0:128, :])
                nc.sync.dma_start(t_wo[1][:], wo[128:256, :])
                nc.sync.dma_start(t_vm[:], vmask[:])
                # V projection: two half-column passes of 8 k-tiles
                for g in range(2):
                    accs = [pp.tile([128, 256], F32, tag="acc",
                                    name=f"accv_{i}") for i in range(8)]
                    for c in range(NDC):
                        xt = xpool.tile([128, 1024], BF16, tag="xtv")
                        nc.sync.dma_start(
                            xt[:], xTv[c * 128:(c + 1) * 128,
                                       g * 1024:(g + 1) * 1024])
                        for kt8 in range(8):
                            nc.tensor.matmul(
                                accs[kt8][:],
                                xt[:, kt8 * 128:(kt8 + 1) * 128],
                                t_wv[:, c * 256:(c + 1) * 256],
                                start=(c == 0), stop=(c == NDC - 1))
                    for kt8 in range(8):
                        t = g * 8 + kt8
                        # [128, 4, 64] strided copy: slot j -> V1 block
                        src = accs[kt8][:].rearrange("p (j c) -> p j c", c=64)
                        dst = t_v1[:, t * 260:(t + 1) * 260].rearrange(
                            "p (j c) -> p j c", c=65)[:, :, 0:64]
                        with nc.allow_low_precision(reason="f32r 4B"):
                            nc.vector.tensor_copy(dst, src)

            # ---- phase A: attention with fused output projection ----
            with tc.tile_pool(name="ap", bufs=1, space="PSUM") as ap:
                def emit_outproj(q, qts=None):
                    # output projection for q-tiles of chunk q; accumulate
                    # pair 1 (shorter-plan slots) first so the first o_ps
                    # matmul fires before pair 0 is normalized
                    if qts is None:
                        qts = range(q * (QC // 128), (q + 1) * (QC // 128))
                    for qt in qts:
                        ts = slice(qt * 128, (qt + 1) * 128)
                        stage = opool.tile([128, D], F32, tag="ostage")
                        for ch in range(2):
                            o_ps = ap.tile([128, 512], F32, tag="sT", bufs=2)
                            for p2 in (1, 0):
                                nc.tensor.matmul(
                                    o_ps[:], t_pb[p2][:, ts],
                                    t_wo[p2][:, ch * 512:(ch + 1) * 512],
                                    start=(p2 == 1), stop=(p2 == 0))
                            nc.any.tensor_copy(
                                stage[:, ch * 512:(ch + 1) * 512], o_ps[:])
                        nc.sync.dma_start(out[ts, :], stage[:])

                for q in range(N // QC):
                    qs = slice(q * QC, (q + 1) * QC)
                    accs2 = [ap.tile([65, QC], F32, tag="acc2", bufs=4,
                                     name=f"acc_{j}") for j in range(HPC)]
                    # round-robin the exp-batch groups across slots so the
                    # in-flight S->exp->PV chains are independent and PE
                    # always has a matmul ready while ScalarE runs exp
                    sched = []
                    for j in range(HPC):
                        for gi, g in enumerate(plans[j]):
                            sched.append((j, gi, g))
                    sched.sort(key=lambda x: (x[1], x[0]))
                    for gidx, (j, gi, (t0, nt)) in enumerate(sched):
                        if gidx == 4 and q > 0:
                            # previous chunk's output projection, emitted a
                            # few groups in so its PSUM slots don't block
                            # this chunk's S matmuls during the norm tail
                            emit_outproj(q - 1)
                        p, half = j // 2, j % 2
                        rows = slice(half * 64, (half + 1) * 64)
                        acc = accs2[j]
                        sT = ap.tile([128, nt * QC], F32, tag="sT", bufs=2)
                        for i in range(nt):
                            t = t0 + i
                            nc.tensor.matmul(
                                sT[:, i * QC:(i + 1) * QC],
                                t_kT[p][rows, t * 128:(t + 1) * 128],
                                t_qT[p][rows, qs],
                                start=True, stop=True)
                        pT = ptpool.tile([128, nt * QC], BF16, tag="pT")
                        nc.scalar.activation(
                            pT[:], sT[:], AF.Exp, scale=0.125,
                            bias=t_vm[:, j * NKT + t0: j * NKT + t0 + 1])
                        for i in range(nt):
                            t = t0 + i
                            base = (t * HPC + j) * 65
                            nc.tensor.matmul(
                                acc[:], t_v1[:, base: base + 65],
                                pT[:, i * QC:(i + 1) * QC],
                                start=(gi == 0 and i == 0),
                                stop=(gi == len(plans[j]) - 1 and i == nt - 1))
                        if gi == len(plans[j]) - 1:
                            # normalize this slot as soon as its last PV is
                            # in: 1/denom (DVE), partition-broadcast
                            # (GpSimd), scale (DVE)
                            r1 = nrmpool.tile([1, QC], F32, tag="r1")
                            nc.vector.reciprocal(r1[:], acc[64:65, :])
                            bc_sb = nrmpool.tile([64, QC], F32, tag="bc_sb")
                            nc.gpsimd.partition_broadcast(bc_sb[:], r1[:])
                            with nc.allow_low_precision(reason="f32r 4B"):
                                nc.vector.tensor_mul(
                                    t_pb[p][rows, qs], acc[0:64, :],
                                    bc_sb[:])
                emit_outproj(N // QC - 1)

    nc.finalize()
    return nc


def _make_plans(trips, vls_by_slot):
    """Greedy pair batching: (t, t+1) share one exp iff every core's vl is
    outside the open interval (128*t, 128*(t+2)) - then one bias column
    describes both tiles on every core."""
    plans = []
    for j in range(HPC):
        plan, t = [], 0
        while t < trips[j]:
            if t + 1 < trips[j] and all(
                    v <= 128 * t or v >= 128 * (t + 2)
                    for v in vls_by_slot[j]):
                plan.append((t, 2))
                t += 2
            else:
                plan.append((t, 1))
                t += 1
        plans.append(plan)
    return plans


def kernel(queries, keys, values, valid_len, Wq, Wk, Wv, Wo):
    global LAST_RESULTS
    queries = np.asarray(queries, dtype=np.float32)
    keys = np.asarray(keys, dtype=np.float32)
    values = np.asarray(values, dtype=np.float32)
    Wq = np.asarray(Wq, dtype=np.float32)
    Wk = np.asarray(Wk, dtype=np.float32)
    Wv = np.asarray(Wv, dtype=np.float32)
    Wo = np.asarray(Wo, dtype=np.float32)
    vl = np.asarray(valid_len).astype(np.int64).reshape(B * H)

    # rank-aligned slot assignment: per batch, heads sorted by vl desc;
    # slot j of the 4 cores of that batch takes ranks 4j..4j+3
    order = {}
    for b in range(B):
        idx = (np.argsort(-vl[b * H:(b + 1) * H], kind="stable") + b * H)
        for cg in range(4):
            order[b * 4 + cg] = [int(idx[4 * j + cg]) for j in range(HPC)]
    trips, vls_by_slot = [], []
    for j in range(HPC):
        vs = [int(vl[order[c][j]]) for c in range(NCORES)]
        vls_by_slot.append(vs)
        m = max(-(-v // 128) for v in vs)
        trips.append(max(1, min(NKT, m)))
    plans = _make_plans(trips, vls_by_slot)

    nc = _build_program(tuple(trips), plans)

    in_maps = []
    for c in range(NCORES):
        b = c // 4
        heads = order[c]
        cols = np.concatenate(
            [np.arange((h - b * H) * DH, (h - b * H + 1) * DH) for h in heads])

        def wlayout(w):
            return np.ascontiguousarray(
                w[:, cols].reshape(NDC, 128, 256).transpose(1, 0, 2)
                .reshape(128, NDC * 256).astype(NPBF16))

        vm = np.zeros((128, HPC * NKT), np.float32)
        for j, h in enumerate(heads):
            bias = np.where(np.arange(N) < vl[h], 0.0, MASK_BIAS)
            vm[:, j * NKT:(j + 1) * NKT] = bias.reshape(NKT, 128).T

        in_maps.append({
            "xTq": np.ascontiguousarray(queries[b].T.astype(NPBF16)),
            "xTk": np.ascontiguousarray(keys[b].T.astype(NPBF16)),
            "xTv": np.ascontiguousarray(values[b].T.astype(NPBF16)),
            "wq": wlayout(Wq),
            "wk": wlayout(Wk),
            "wv": wlayout(Wv),
            "wo": np.ascontiguousarray(Wo[cols, :]).astype(NPBF16),
            "vmask": vm,
        })

    LAST_RESULTS = run_bass_kernel_spmd(nc, in_maps, list(range(NCORES)))
    res = LAST_RESULTS.results

    out = np.zeros((B, N, D), np.float64)
    for c in range(NCORES):
        out[c // 4] += res[c]["out"].astype(np.float64)
    return out.astype(np.float32)


# revision 36
# speedup vs baseline: 1.0138x; 1.0094x over previous
"""Multi-head attention TRN2 kernel (8 NeuronCores, SPMD).

Problem: B=2, N=2048, D=1024, H=16 heads of dim 64, fp32, per-(b,h)
key-length masking (valid_len, length 32).

Sharding: batch*heads across 8 cores - core c handles batch b=c//4 and 4
heads ("slots", rank-aligned by valid_len so the SPMD trip counts stay
balanced).  Per core:

  phase P (projections, bf16 inputs to halve HBM traffic):
    K^T/Q^T = Wslice^T @ x^T   (head dims on partitions, positions free)
    V       = x^T-tiles as lhsT, Wv as rhs  (positions on partitions),
              copied into V1 = [V_j | 1] blocks per (key-tile, slot)
  phase A (attention, all-bf16 operands, f32 PSUM accumulate):
    S^T   = K^T.T @ Q^T per (slot, key-tile), batched in PSUM pairs
    P^T   = exp(S^T/8 + bias) on ScalarE - the valid_len mask is a
            per-partition bias column (0 / -30000), so no V masking ops;
            key-tile pairs that are uniformly valid/invalid across all
            cores share one exp instruction ([128,1024] batch); groups
            are emitted round-robin across slots so independent
            S->exp->PV chains keep the tensor engine dense
    acc   = V1.T @ P^T accumulated over key tiles (ones column gives the
            softmax denominator as row 64)
    normalize per slot right after its last PV: DVE bit-exact
    reciprocal, GpSimd partition broadcast, DVE scale
    out_partial = heads^T.T @ Wo_slice -> (2048, 1024) per core; each
    chunk's output projection is emitted a few groups into the NEXT
    chunk so its PSUM slots never starve the S-matmul pipeline
Host sums the 4 per-core partials of each batch element (the unshard for
the row-sharded Wo) and gathers.
"""
import sys
import numpy as np
from contextlib import ExitStack

sys.path.insert(0, "/opt/trn_rl_repo")

import concourse.bass as bass  # noqa: E402
from concourse import bacc, mybir  # noqa: E402
import concourse.tile as tile  # noqa: E402
from concourse.bass_utils import run_bass_kernel_spmd  # noqa: E402

F32 = mybir.dt.float32
F32R = mybir.dt.float32r
BF16 = mybir.dt.bfloat16
AF = mybir.ActivationFunctionType
NPBF16 = mybir.dt.np(BF16)

B, N, D, H = 2, 2048, 1024, 16
DH = 64
HPC = 4          # heads (slots) per core
NCORES = 8
QC = 512         # q chunk (matmul free dim)
NKT = N // 128   # 16 k tiles
NDC = D // 128   # 8 contraction chunks
MASK_BIAS = -30000.0

LAST_RESULTS = None  # BassKernelResults of the most recent run (for tooling)


def _build_program(trips, plans):
    """trips: 4 ints (k-tile count per slot); plans: per slot, list of
    (t0, ntiles) exp-batch groups covering range(trips[j])."""
    nc = bacc.Bacc("TRN2", target_bir_lowering=False, debug=False,
                   num_devices=NCORES)

    xTq = nc.dram_tensor("xTq", [D, N], BF16, kind="ExternalInput")
    xTk = nc.dram_tensor("xTk", [D, N], BF16, kind="ExternalInput")
    xTv = nc.dram_tensor("xTv", [D, N], BF16, kind="ExternalInput")
    wq = nc.dram_tensor("wq", [128, NDC * 256], BF16, kind="ExternalInput")
    wk = nc.dram_tensor("wk", [128, NDC * 256], BF16, kind="ExternalInput")
    wv = nc.dram_tensor("wv", [128, NDC * 256], BF16, kind="ExternalInput")
    wo = nc.dram_tensor("wo", [256, D], BF16, kind="ExternalInput")
    vmask = nc.dram_tensor("vmask", [128, HPC * NKT], F32, kind="ExternalInput")
    out = nc.dram_tensor("out", [N, D], F32, kind="ExternalOutput")

    with tile.TileContext(nc) as tc:
        with ExitStack() as ctx:
            wpool = ctx.enter_context(tc.tile_pool(name="wpool", bufs=1))
            xpool = ctx.enter_context(tc.tile_pool(name="xpool", bufs=6))
            qkpool = ctx.enter_context(tc.tile_pool(name="qkpool", bufs=1))
            v1pool = ctx.enter_context(tc.tile_pool(name="v1pool", bufs=1))
            ptpool = ctx.enter_context(tc.tile_pool(name="ptpool", bufs=8))
            nrmpool = ctx.enter_context(tc.tile_pool(name="nrmpool", bufs=4))
            pbpool = ctx.enter_context(tc.tile_pool(name="pbpool", bufs=1))
            opool = ctx.enter_context(tc.tile_pool(name="opool", bufs=8))

            # only wk is needed before the first matmul; the other weight
            # loads are issued just before their consuming phase so they
            # don't delay the first xk chunks
            t_wk = wpool.tile([128, NDC * 256], BF16, tag="wk")
            t_wq = wpool.tile([128, NDC * 256], BF16, tag="wq")
            t_wv = wpool.tile([128, NDC * 256], BF16, tag="wv")
            t_wo = [wpool.tile([128, D], BF16, tag=f"wo{p}", name=f"t_wo{p}")
                    for p in range(2)]
            t_vm = wpool.tile([128, HPC * NKT], F32, tag="vm")

            # K^T/Q^T: [128 dims (2 slots), N] per slot-pair
            t_kT = [qkpool.tile([128, N], BF16, tag=f"kT{p}", name=f"t_kT{p}")
                    for p in range(2)]
            t_qT = [qkpool.tile([128, N], BF16, tag=f"qT{p}", name=f"t_qT{p}")
                    for p in range(2)]
            # V1: per key-tile t, 4 blocks of [V_j (64 cols) | ones (1 col)]
            t_v1 = v1pool.tile([128, NKT * HPC * 65], BF16, tag="v1")
            # normalized heads^T per slot pair: [128 dims, N]
            t_pb = [pbpool.tile([128, N], BF16, tag=f"pb{p}", name=f"t_pb{p}")
                    for p in range(2)]

            # ones columns of V1, one strided memset
            ones_ap = t_v1[:].rearrange(
                "p (b c) -> p b c", c=65)[:, :, 64:65]
            nc.vector.memset(ones_ap, 1.0)

            # ---- phase P: projections (K, Q, V) ----
            with tc.tile_pool(name="pp", bufs=8, space="PSUM") as pp:
                for si, (xin, wsb, dsts) in enumerate(
                        ((xTk, t_wk, t_kT), (xTq, t_wq, t_qT))):
                    accs = [pp.tile([128, QC], F32, tag="acc", name=f"acc_{i}")
                            for i in range(8)]
                    for c in range(NDC):
                        xt = xpool.tile([128, N], BF16, tag="xt")
                        nc.sync.dma_start(xt[:], xin[c * 128:(c + 1) * 128, :])
                        if si == 0:
                            # wk sliced per chunk: the first matmul only
                            # gates on 64KB of weights + one x chunk
                            nc.sync.dma_start(
                                t_wk[:, c * 256:(c + 1) * 256],
                                wk[:, c * 256:(c + 1) * 256])
                        for m in range(2):
                            for qq in range(4):
                                nc.tensor.matmul(
                                    accs[m * 4 + qq][:],
                                    wsb[:, c * 256 + m * 128:
                                        c * 256 + (m + 1) * 128],
                                    xt[:, qq * QC:(qq + 1) * QC],
                                    start=(c == 0), stop=(c == NDC - 1))
                    if si == 0:
                        nc.sync.dma_start(t_wq[:], wq[:])
                    else:
                        nc.sync.dma_start(t_wv[:], wv[:])
                    # drain PSUM->SBUF casts on BOTH ScalarE and DVE, q-chunk
                    # 0 first: the first attention S matmuls gate only on the
                    # chunk-0 casts, so phase A starts ~2.5us earlier
                    for n, i in enumerate((0, 4, 1, 5, 2, 6, 3, 7)):
                        dst = dsts[i // 4][:, (i % 4) * QC:(i % 4 + 1) * QC]
                        with nc.allow_low_precision(reason="f32r 4B"):
                            if n % 2 == 0:
                                nc.scalar.activation(dst, accs[i][:], AF.Copy)
                            else:
                                nc.vector.tensor_copy(dst, accs[i][:])
                nc.sync.dma_start(t_wo[0][:], wo[0:128, :])
                nc.sync.dma_start(t_wo[1][:], wo[128:256, :])
                nc.sync.dma_start(t_vm[:], vmask[:])
                # V projection: two half-column passes of 8 k-tiles
                for g in range(2):
                    accs = [pp.tile([128, 256], F32, tag="acc",
                                    name=f"accv_{i}") for i in range(8)]
                    for c in range(NDC):
                        xt = xpool.tile([128, 1024], BF16, tag="xtv")
                        nc.sync.dma_start(
                            xt[:], xTv[c * 128:(c + 1) * 128,
                                       g * 1024:(g + 1) * 1024])
                        for kt8 in range(8):
                            nc.tensor.matmul(
                                accs[kt8][:],
                                xt[:, kt8 * 128:(kt8 + 1) * 128],
                                t_wv[:, c * 256:(c + 1) * 256],
                                start=(c == 0), stop=(c == NDC - 1))
                    for kt8 in range(8):
                        t = g * 8 + kt8
                        # [128, 4, 64] strided copy: slot j -> V1 block
                        src = accs[kt8][:].rearrange("p (j c) -> p j c", c=64)
                        dst = t_v1[:, t * 260:(t + 1) * 260].rearrange(
                            "p (j c) -> p j c", c=65)[:, :, 0:64]
                        with nc.allow_low_precision(reason="f32r 4B"):
                            nc.vector.tensor_copy(dst, src)

            # ---- phase A: attention with fused output projection ----
            with tc.tile_pool(name="ap", bufs=1, space="PSUM") as ap:
                def emit_outproj(q, qts=None):
                    # output projection for q-tiles of chunk q; accumulate
                    # pair 1 (shorter-plan slots) first so the first o_ps
                    # matmul fires before pair 0 is normalized
                    if qts is None:
                        qts = range(q * (QC // 128), (q + 1) * (QC // 128))
                    for qt in qts:
                        ts = slice(qt * 128, (qt + 1) * 128)
                        stage = opool.tile([128, D], F32, tag="ostage")
                        for ch in range(2):
                            o_ps = ap.tile([128, 512], F32, tag="sT", bufs=2)
                            for p2 in (1, 0):
                                nc.tensor.matmul(
                                    o_ps[:], t_pb[p2][:, ts],
                                    t_wo[p2][:, ch * 512:(ch + 1) * 512],
                                    start=(p2 == 1), stop=(p2 == 0))
                            nc.vector.tensor_copy(
                                stage[:, ch * 512:(ch + 1) * 512], o_ps[:])
                        nc.sync.dma_start(out[ts, :], stage[:])

                for q in range(N // QC):
                    qs = slice(q * QC, (q + 1) * QC)
                    accs2 = [ap.tile([65, QC], F32, tag="acc2", bufs=4,
                                     name=f"acc_{j}") for j in range(HPC)]
                    # round-robin the exp-batch groups across slots so the
                    # in-flight S->exp->PV chains are independent and PE
                    # always has a matmul ready while ScalarE runs exp
                    sched = []
                    for j in range(HPC):
                        for gi, g in enumerate(plans[j]):
                            sched.append((j, gi, g))
                    sched.sort(key=lambda x: (x[1], x[0]))
                    for gidx, (j, gi, (t0, nt)) in enumerate(sched):
                        if gidx == 4 and q > 0:
                            # previous chunk's output projection, emitted a
                            # few groups in so its PSUM slots don't block
                            # this chunk's S matmuls during the norm tail
                            emit_outproj(q - 1)
                        p, half = j // 2, j % 2
                        rows = slice(half * 64, (half + 1) * 64)
                        acc = accs2[j]
                        sT = ap.tile([128, nt * QC], F32, tag="sT", bufs=2)
                        for i in range(nt):
                            t = t0 + i
                            nc.tensor.matmul(
                                sT[:, i * QC:(i + 1) * QC],
                                t_kT[p][rows, t * 128:(t + 1) * 128],
                                t_qT[p][rows, qs],
                                start=True, stop=True)
                        pT = ptpool.tile([128, nt * QC], BF16, tag="pT")
                        nc.scalar.activation(
                            pT[:], sT[:], AF.Exp, scale=0.125,
                            bias=t_vm[:, j * NKT + t0: j * NKT + t0 + 1])
                        for i in range(nt):
                            t = t0 + i
                            base = (t * HPC + j) * 65
                            nc.tensor.matmul(
                                acc[:], t_v1[:, base: base + 65],
                                pT[:, i * QC:(i + 1) * QC],
                                start=(gi == 0 and i == 0),
                                stop=(gi == len(plans[j]) - 1 and i == nt - 1))
                        if gi == len(plans[j]) - 1:
                            # normalize this slot as soon as its last PV is
                            # in: 1/denom (DVE), partition-broadcast
                            # (GpSimd), scale (DVE)
                            r1 = nrmpool.tile([1, QC], F32, tag="r1")
                            nc.vector.reciprocal(r1[:], acc[64:65, :])
                            bc_sb = nrmpool.tile([64, QC], F32, tag="bc_sb")
                            nc.gpsimd.partition_broadcast(bc_sb[:], r1[:])
                            with nc.allow_low_precision(reason="f32r 4B"):
                                nc.vector.tensor_mul(
                                    t_pb[p][rows, qs], acc[0:64, :],
                                    bc_sb[:])
                emit_outproj(N // QC - 1)

    nc.finalize()
    return nc


def _make_plans(trips, vls_by_slot):
    """Greedy pair batching: (t, t+1) share one exp iff every core's vl is
    outside the open interval (128*t, 128*(t+2)) - then one bias column
    describes both tiles on every core."""
    plans = []
    for j in range(HPC):
        plan, t = [], 0
        while t < trips[j]:
            if t + 1 < trips[j] and all(
                    v <= 128 * t or v >= 128 * (t + 2)
                    for v in vls_by_slot[j]):
                plan.append((t, 2))
                t += 2
            else:
                plan.append((t, 1))
                t += 1
        plans.append(plan)
    return plans


def kernel(queries, keys, values, valid_len, Wq, Wk, Wv, Wo):
    global LAST_RESULTS
    queries = np.asarray(queries, dtype=np.float32)
    keys = np.asarray(keys, dtype=np.float32)
    values = np.asarray(values, dtype=np.float32)
    Wq = np.asarray(Wq, dtype=np.float32)
    Wk = np.asarray(Wk, dtype=np.float32)
    Wv = np.asarray(Wv, dtype=np.float32)
    Wo = np.asarray(Wo, dtype=np.float32)
    vl = np.asarray(valid_len).astype(np.int64).reshape(B * H)

    # rank-aligned slot assignment: per batch, heads sorted by vl desc;
    # slot j of the 4 cores of that batch takes ranks 4j..4j+3
    order = {}
    for b in range(B):
        idx = (np.argsort(-vl[b * H:(b + 1) * H], kind="stable") + b * H)
        for cg in range(4):
            order[b * 4 + cg] = [int(idx[4 * j + cg]) for j in range(HPC)]
    trips, vls_by_slot = [], []
    for j in range(HPC):
        vs = [int(vl[order[c][j]]) for c in range(NCORES)]
        vls_by_slot.append(vs)
        m = max(-(-v // 128) for v in vs)
        trips.append(max(1, min(NKT, m)))
    plans = _make_plans(trips, vls_by_slot)

    nc = _build_program(tuple(trips), plans)

    in_maps = []
    for c in range(NCORES):
        b = c // 4
        heads = order[c]
        cols = np.concatenate(
            [np.arange((h - b * H) * DH, (h - b * H + 1) * DH) for h in heads])

        def wlayout(w):
            return np.ascontiguousarray(
                w[:, cols].reshape(NDC, 128, 256).transpose(1, 0, 2)
                .reshape(128, NDC * 256).astype(NPBF16))

        vm = np.zeros((128, HPC * NKT), np.float32)
        for j, h in enumerate(heads):
            bias = np.where(np.arange(N) < vl[h], 0.0, MASK_BIAS)
            vm[:, j * NKT:(j + 1) * NKT] = bias.reshape(NKT, 128).T

        in_maps.append({
            "xTq": np.ascontiguousarray(queries[b].T.astype(NPBF16)),
            "xTk": np.ascontiguousarray(keys[b].T.astype(NPBF16)),
            "xTv": np.ascontiguousarray(values[b].T.astype(NPBF16)),
            "wq": wlayout(Wq),
            "wk": wlayout(Wk),
            "wv": wlayout(Wv),
            "wo": np.ascontiguousarray(Wo[cols, :]).astype(NPBF16),
            "vmask": vm,
        })

    LAST_RESULTS = run_bass_kernel_spmd(nc, in_maps, list(range(NCORES)))
    res = LAST_RESULTS.results

    out = np.zeros((B, N, D), np.float64)
    for c in range(NCORES):
        out[c // 4] += res[c]["out"].astype(np.float64)
    return out.astype(np.float32)


# revision 37
# speedup vs baseline: 1.0214x; 1.0075x over previous
"""Multi-head attention TRN2 kernel (8 NeuronCores, SPMD).

Problem: B=2, N=2048, D=1024, H=16 heads of dim 64, fp32, per-(b,h)
key-length masking (valid_len, length 32).

Sharding: batch*heads across 8 cores - core c handles batch b=c//4 and 4
heads ("slots", rank-aligned by valid_len so the SPMD trip counts stay
balanced).  Per core:

  phase P (projections, bf16 inputs to halve HBM traffic):
    K^T/Q^T = Wslice^T @ x^T   (head dims on partitions, positions free)
    V       = x^T-tiles as lhsT, Wv as rhs  (positions on partitions),
              copied into V1 = [V_j | 1] blocks per (key-tile, slot)
  phase A (attention, all-bf16 operands, f32 PSUM accumulate):
    S^T   = K^T.T @ Q^T per (slot, key-tile), batched in PSUM pairs
    P^T   = exp(S^T/8 + bias) on ScalarE - the valid_len mask is a
            per-partition bias column (0 / -30000), so no V masking ops;
            key-tile pairs that are uniformly valid/invalid across all
            cores share one exp instruction ([128,1024] batch); groups
            are emitted round-robin across slots so independent
            S->exp->PV chains keep the tensor engine dense
    acc   = V1.T @ P^T accumulated over key tiles (ones column gives the
            softmax denominator as row 64)
    normalize per slot right after its last PV: DVE bit-exact
    reciprocal, GpSimd partition broadcast, DVE scale
    out_partial = heads^T.T @ Wo_slice -> (2048, 1024) per core; each
    chunk's output projection is emitted a few groups into the NEXT
    chunk so its PSUM slots never starve the S-matmul pipeline
Host sums the 4 per-core partials of each batch element (the unshard for
the row-sharded Wo) and gathers.
"""
import sys
import numpy as np
from contextlib import ExitStack

sys.path.insert(0, "/opt/trn_rl_repo")

import concourse.bass as bass  # noqa: E402
from concourse import bacc, mybir  # noqa: E402
import concourse.tile as tile  # noqa: E402
from concourse.bass_utils import run_bass_kernel_spmd  # noqa: E402

F32 = mybir.dt.float32
F32R = mybir.dt.float32r
BF16 = mybir.dt.bfloat16
AF = mybir.ActivationFunctionType
NPBF16 = mybir.dt.np(BF16)

B, N, D, H = 2, 2048, 1024, 16
DH = 64
HPC = 4          # heads (slots) per core
NCORES = 8
QC = 512         # q chunk (matmul free dim)
NKT = N // 128   # 16 k tiles
NDC = D // 128   # 8 contraction chunks
MASK_BIAS = -30000.0

LAST_RESULTS = None  # BassKernelResults of the most recent run (for tooling)


def _build_program(trips, plans):
    """trips: 4 ints (k-tile count per slot); plans: per slot, list of
    (t0, ntiles) exp-batch groups covering range(trips[j])."""
    nc = bacc.Bacc("TRN2", target_bir_lowering=False, debug=False,
                   num_devices=NCORES)

    xTq = nc.dram_tensor("xTq", [D, N], BF16, kind="ExternalInput")
    xTk = nc.dram_tensor("xTk", [D, N], BF16, kind="ExternalInput")
    xTv = nc.dram_tensor("xTv", [D, N], BF16, kind="ExternalInput")
    wq = nc.dram_tensor("wq", [128, NDC * 256], BF16, kind="ExternalInput")
    wk = nc.dram_tensor("wk", [128, NDC * 256], BF16, kind="ExternalInput")
    wv = nc.dram_tensor("wv", [128, NDC * 256], BF16, kind="ExternalInput")
    wo = nc.dram_tensor("wo", [256, D], BF16, kind="ExternalInput")
    vmask = nc.dram_tensor("vmask", [128, HPC * NKT], F32, kind="ExternalInput")
    out = nc.dram_tensor("out", [N, D], BF16, kind="ExternalOutput")

    with tile.TileContext(nc) as tc:
        with ExitStack() as ctx:
            wpool = ctx.enter_context(tc.tile_pool(name="wpool", bufs=1))
            xpool = ctx.enter_context(tc.tile_pool(name="xpool", bufs=6))
            qkpool = ctx.enter_context(tc.tile_pool(name="qkpool", bufs=1))
            v1pool = ctx.enter_context(tc.tile_pool(name="v1pool", bufs=1))
            ptpool = ctx.enter_context(tc.tile_pool(name="ptpool", bufs=8))
            nrmpool = ctx.enter_context(tc.tile_pool(name="nrmpool", bufs=4))
            pbpool = ctx.enter_context(tc.tile_pool(name="pbpool", bufs=1))
            opool = ctx.enter_context(tc.tile_pool(name="opool", bufs=8))

            # only wk is needed before the first matmul; the other weight
            # loads are issued just before their consuming phase so they
            # don't delay the first xk chunks
            t_wk = wpool.tile([128, NDC * 256], BF16, tag="wk")
            t_wq = wpool.tile([128, NDC * 256], BF16, tag="wq")
            t_wv = wpool.tile([128, NDC * 256], BF16, tag="wv")
            t_wo = [wpool.tile([128, D], BF16, tag=f"wo{p}", name=f"t_wo{p}")
                    for p in range(2)]
            t_vm = wpool.tile([128, HPC * NKT], F32, tag="vm")

            # K^T/Q^T: [128 dims (2 slots), N] per slot-pair
            t_kT = [qkpool.tile([128, N], BF16, tag=f"kT{p}", name=f"t_kT{p}")
                    for p in range(2)]
            t_qT = [qkpool.tile([128, N], BF16, tag=f"qT{p}", name=f"t_qT{p}")
                    for p in range(2)]
            # V1: per key-tile t, 4 blocks of [V_j (64 cols) | ones (1 col)]
            t_v1 = v1pool.tile([128, NKT * HPC * 65], BF16, tag="v1")
            # normalized heads^T per slot pair: [128 dims, N]
            t_pb = [pbpool.tile([128, N], BF16, tag=f"pb{p}", name=f"t_pb{p}")
                    for p in range(2)]

            # ones columns of V1, one strided memset
            ones_ap = t_v1[:].rearrange(
                "p (b c) -> p b c", c=65)[:, :, 64:65]
            nc.vector.memset(ones_ap, 1.0)

            # ---- phase P: projections (K, Q, V) ----
            with tc.tile_pool(name="pp", bufs=8, space="PSUM") as pp:
                for si, (xin, wsb, dsts) in enumerate(
                        ((xTk, t_wk, t_kT), (xTq, t_wq, t_qT))):
                    accs = [pp.tile([128, QC], F32, tag="acc", name=f"acc_{i}")
                            for i in range(8)]
                    for c in range(NDC):
                        xt = xpool.tile([128, N], BF16, tag="xt")
                        nc.sync.dma_start(xt[:], xin[c * 128:(c + 1) * 128, :])
                        if si == 0:
                            # wk sliced per chunk: the first matmul only
                            # gates on 64KB of weights + one x chunk
                            nc.sync.dma_start(
                                t_wk[:, c * 256:(c + 1) * 256],
                                wk[:, c * 256:(c + 1) * 256])
                        for m in range(2):
                            for qq in range(4):
                                nc.tensor.matmul(
                                    accs[m * 4 + qq][:],
                                    wsb[:, c * 256 + m * 128:
                                        c * 256 + (m + 1) * 128],
                                    xt[:, qq * QC:(qq + 1) * QC],
                                    start=(c == 0), stop=(c == NDC - 1))
                    if si == 0:
                        nc.sync.dma_start(t_wq[:], wq[:])
                    else:
                        nc.sync.dma_start(t_wv[:], wv[:])
                    # drain PSUM->SBUF casts on BOTH ScalarE and DVE, q-chunk
                    # 0 first: the first attention S matmuls gate only on the
                    # chunk-0 casts, so phase A starts ~2.5us earlier
                    for n, i in enumerate((0, 4, 1, 5, 2, 6, 3, 7)):
                        dst = dsts[i // 4][:, (i % 4) * QC:(i % 4 + 1) * QC]
                        with nc.allow_low_precision(reason="f32r 4B"):
                            if n % 2 == 0:
                                nc.scalar.activation(dst, accs[i][:], AF.Copy)
                            else:
                                nc.vector.tensor_copy(dst, accs[i][:])
                nc.sync.dma_start(t_wo[0][:], wo[0:128, :])
                nc.sync.dma_start(t_wo[1][:], wo[128:256, :])
                nc.sync.dma_start(t_vm[:], vmask[:])
                # V projection: two half-column passes of 8 k-tiles
                for g in range(2):
                    accs = [pp.tile([128, 256], F32, tag="acc",
                                    name=f"accv_{i}") for i in range(8)]
                    for c in range(NDC):
                        xt = xpool.tile([128, 1024], BF16, tag="xtv")
                        nc.sync.dma_start(
                            xt[:], xTv[c * 128:(c + 1) * 128,
                                       g * 1024:(g + 1) * 1024])
                        for kt8 in range(8):
                            nc.tensor.matmul(
                                accs[kt8][:],
                                xt[:, kt8 * 128:(kt8 + 1) * 128],
                                t_wv[:, c * 256:(c + 1) * 256],
                                start=(c == 0), stop=(c == NDC - 1))
                    for kt8 in range(8):
                        t = g * 8 + kt8
                        # [128, 4, 64] strided copy: slot j -> V1 block
                        src = accs[kt8][:].rearrange("p (j c) -> p j c", c=64)
                        dst = t_v1[:, t * 260:(t + 1) * 260].rearrange(
                            "p (j c) -> p j c", c=65)[:, :, 0:64]
                        with nc.allow_low_precision(reason="f32r 4B"):
                            nc.vector.tensor_copy(dst, src)

            # ---- phase A: attention with fused output projection ----
            with tc.tile_pool(name="ap", bufs=1, space="PSUM") as ap:
                def emit_outproj(q, qts=None):
                    # output projection for q-tiles of chunk q; accumulate
                    # pair 1 (shorter-plan slots) first so the first o_ps
                    # matmul fires before pair 0 is normalized
                    if qts is None:
                        qts = range(q * (QC // 128), (q + 1) * (QC // 128))
                    for qt in qts:
                        ts = slice(qt * 128, (qt + 1) * 128)
                        stage = opool.tile([128, D], BF16, tag="ostage")
                        for ch in range(2):
                            o_ps = ap.tile([128, 512], F32, tag="sT", bufs=2)
                            for p2 in (1, 0):
                                nc.tensor.matmul(
                                    o_ps[:], t_pb[p2][:, ts],
                                    t_wo[p2][:, ch * 512:(ch + 1) * 512],
                                    start=(p2 == 1), stop=(p2 == 0))
                            with nc.allow_low_precision(reason="bf16 out"):
                                nc.vector.tensor_copy(
                                    stage[:, ch * 512:(ch + 1) * 512],
                                    o_ps[:])
                        nc.sync.dma_start(out[ts, :], stage[:])

                for q in range(N // QC):
                    qs = slice(q * QC, (q + 1) * QC)
                    accs2 = [ap.tile([65, QC], F32, tag="acc2", bufs=4,
                                     name=f"acc_{j}") for j in range(HPC)]
                    # round-robin the exp-batch groups across slots so the
                    # in-flight S->exp->PV chains are independent and PE
                    # always has a matmul ready while ScalarE runs exp
                    sched = []
                    for j in range(HPC):
                        for gi, g in enumerate(plans[j]):
                            sched.append((j, gi, g))
                    sched.sort(key=lambda x: (x[1], x[0]))
                    for gidx, (j, gi, (t0, nt)) in enumerate(sched):
                        if gidx == 4 and q > 0:
                            # previous chunk's output projection, emitted a
                            # few groups in so its PSUM slots don't block
                            # this chunk's S matmuls during the norm tail
                            emit_outproj(q - 1)
                        p, half = j // 2, j % 2
                        rows = slice(half * 64, (half + 1) * 64)
                        acc = accs2[j]
                        sT = ap.tile([128, nt * QC], F32, tag="sT", bufs=2)
                        for i in range(nt):
                            t = t0 + i
                            nc.tensor.matmul(
                                sT[:, i * QC:(i + 1) * QC],
                                t_kT[p][rows, t * 128:(t + 1) * 128],
                                t_qT[p][rows, qs],
                                start=True, stop=True)
                        pT = ptpool.tile([128, nt * QC], BF16, tag="pT")
                        nc.scalar.activation(
                            pT[:], sT[:], AF.Exp, scale=0.125,
                            bias=t_vm[:, j * NKT + t0: j * NKT + t0 + 1])
                        for i in range(nt):
                            t = t0 + i
                            base = (t * HPC + j) * 65
                            nc.tensor.matmul(
                                acc[:], t_v1[:, base: base + 65],
                                pT[:, i * QC:(i + 1) * QC],
                                start=(gi == 0 and i == 0),
                                stop=(gi == len(plans[j]) - 1 and i == nt - 1))
                        if gi == len(plans[j]) - 1:
                            # normalize this slot as soon as its last PV is
                            # in: 1/denom (DVE), partition-broadcast
                            # (GpSimd), scale (DVE)
                            r1 = nrmpool.tile([1, QC], F32, tag="r1")
                            nc.vector.reciprocal(r1[:], acc[64:65, :])
                            bc_sb = nrmpool.tile([64, QC], F32, tag="bc_sb")
                            nc.gpsimd.partition_broadcast(bc_sb[:], r1[:])
                            with nc.allow_low_precision(reason="f32r 4B"):
                                nc.vector.tensor_mul(
                                    t_pb[p][rows, qs], acc[0:64, :],
                                    bc_sb[:])
                emit_outproj(N // QC - 1)

    nc.finalize()
    return nc


def _make_plans(trips, vls_by_slot):
    """Greedy pair batching: (t, t+1) share one exp iff every core's vl is
    outside the open interval (128*t, 128*(t+2)) - then one bias column
    describes both tiles on every core."""
    plans = []
    for j in range(HPC):
        plan, t = [], 0
        while t < trips[j]:
            if t + 1 < trips[j] and all(
                    v <= 128 * t or v >= 128 * (t + 2)
                    for v in vls_by_slot[j]):
                plan.append((t, 2))
                t += 2
            else:
                plan.append((t, 1))
                t += 1
        plans.append(plan)
    return plans


def kernel(queries, keys, values, valid_len, Wq, Wk, Wv, Wo):
    global LAST_RESULTS
    queries = np.asarray(queries, dtype=np.float32)
    keys = np.asarray(keys, dtype=np.float32)
    values = np.asarray(values, dtype=np.float32)
    Wq = np.asarray(Wq, dtype=np.float32)
    Wk = np.asarray(Wk, dtype=np.float32)
    Wv = np.asarray(Wv, dtype=np.float32)
    Wo = np.asarray(Wo, dtype=np.float32)
    vl = np.asarray(valid_len).astype(np.int64).reshape(B * H)

    # rank-aligned slot assignment: per batch, heads sorted by vl desc;
    # slot j of the 4 cores of that batch takes ranks 4j..4j+3
    order = {}
    for b in range(B):
        idx = (np.argsort(-vl[b * H:(b + 1) * H], kind="stable") + b * H)
        for cg in range(4):
            order[b * 4 + cg] = [int(idx[4 * j + cg]) for j in range(HPC)]
    trips, vls_by_slot = [], []
    for j in range(HPC):
        vs = [int(vl[order[c][j]]) for c in range(NCORES)]
        vls_by_slot.append(vs)
        m = max(-(-v // 128) for v in vs)
        trips.append(max(1, min(NKT, m)))
    plans = _make_plans(trips, vls_by_slot)

    nc = _build_program(tuple(trips), plans)

    in_maps = []
    for c in range(NCORES):
        b = c // 4
        heads = order[c]
        cols = np.concatenate(
            [np.arange((h - b * H) * DH, (h - b * H + 1) * DH) for h in heads])

        def wlayout(w):
            return np.ascontiguousarray(
                w[:, cols].reshape(NDC, 128, 256).transpose(1, 0, 2)
                .reshape(128, NDC * 256).astype(NPBF16))

        vm = np.zeros((128, HPC * NKT), np.float32)
        for j, h in enumerate(heads):
            bias = np.where(np.arange(N) < vl[h], 0.0, MASK_BIAS)
            vm[:, j * NKT:(j + 1) * NKT] = bias.reshape(NKT, 128).T

        in_maps.append({
            "xTq": np.ascontiguousarray(queries[b].T.astype(NPBF16)),
            "xTk": np.ascontiguousarray(keys[b].T.astype(NPBF16)),
            "xTv": np.ascontiguousarray(values[b].T.astype(NPBF16)),
            "wq": wlayout(Wq),
            "wk": wlayout(Wk),
            "wv": wlayout(Wv),
            "wo": np.ascontiguousarray(Wo[cols, :]).astype(NPBF16),
            "vmask": vm,
        })

    LAST_RESULTS = run_bass_kernel_spmd(nc, in_maps, list(range(NCORES)))
    res = LAST_RESULTS.results

    out = np.zeros((B, N, D), np.float64)
    for c in range(NCORES):
        out[c // 4] += res[c]["out"].astype(np.float64)
    return out.astype(np.float32)
